# revision 7
# baseline (speedup 1.0000x reference)
"""Batched COO SpMM (gnn_message_passing) on TRN2.

out[k, i, :] = sum_{e: row[e]==i} values[k, e] * b[k, col[e], :]
  indices [2, 800000] int32, values [4, 800000] f32, b [4, 50000, 64] f32.

Design: the whole problem runs in ONE device program on ONE NeuronCore.
(Multi-core SPMD dispatch is serialized by the runtime at ~1.5-4.5ms per
device execution, dwarfing the ~0.5ms/core of actual device work — one
execution of an 8x-bigger program is much faster end-to-end.)

Batch-fused element layout: b_t[node, k*64+f] = b[k, node, f] -> 1KB per
node row, so ONE dma_gather descriptor fetches an edge's source features
for all 4 batches. Edges (tokens) are grouped by 128-row output window;
per window, tokens split into col-bank sections (int16 gather index
limit), col-sorted, padded to multiples of 128.

Per chunk: dma_gather from b_t -> gt [128 tokens/col-group, C, 256];
DVE multiplies each token block by its per-batch edge values; per
128-token column DVE builds a one-hot lhsT[t, m] = (relrow[t] == m) and
PE accumulates psum[m,:] += lhsT^T @ gt[:,c,:] over the window's columns.
Window end: PSUM -> SBUF -> DMA to out_t rows. No scatter, no RMW.
Pad tokens: gather node 0, values 0, relrow -1 (one-hot row all zero).
"""
import hashlib

import ml_dtypes
import numpy as np

BF16 = ml_dtypes.bfloat16

N_NODES = 50000
NNZ = 800000
BATCH = 4
FEAT = 64
ELEM = BATCH * FEAT
N_CORES = 1  # whole problem on one core (see module docstring)
ROWS_PER_CORE = N_NODES
BANK = 32768
W = 128  # output rows per PSUM window

_cache = {}


def make_b_t(b):
    """Host-side packing of b into the kernel's batch-fused bf16 layout."""
    b = np.asarray(b, np.float32)
    return np.ascontiguousarray(
        b.transpose(1, 0, 2).reshape(N_NODES, ELEM).astype(BF16)
    )


# ---------------------------------------------------------------- host prep
def _make_structure(rows, cols):
    """Group edges by 128-row output window; per window split into col banks
    (int16 index limit), col-sort, pad to multiples of 128."""
    NW = -(-ROWS_PER_CORE // W)
    win = rows // W
    order = np.argsort(win, kind="stable")
    bounds = np.searchsorted(win[order], np.arange(NW + 1))
    sections = []
    for w in range(NW):
        in_w = order[bounds[w] : bounds[w + 1]]
        cw = cols[in_w]
        a = in_w[cw < BANK]
        b = in_w[cw >= BANK]
        a = a[np.argsort(cols[a], kind="stable")]
        b = b[np.argsort(cols[b], kind="stable")]
        sections.append((a, b))

    chunks = []
    for w in range(NW):
        nA = max(-(-len(sections[w][0]) // 128) * 128, 128)
        nB = -(-len(sections[w][1]) // 128) * 128
        parts = [(w, 0, nA)] + ([(w, 1, nB)] if nB else [])
        for i, (w_, b_, n_) in enumerate(parts):
            chunks.append((w_, b_, n_, i == 0, i == len(parts) - 1))

    g_parts, r_parts, e_parts = [], [], []
    for w, bank_b, n, _, _ in chunks:
        sel = sections[w][bank_b]
        k = len(sel)
        g = np.zeros(n, np.int16)
        rr = np.full(n, -1.0, np.float32)
        e = np.full(n, -1, np.int64)
        g[:k] = (cols[sel] - (BANK if bank_b else 0)).astype(np.int16)
        rr[:k] = (rows[sel] - w * W).astype(np.float32)
        e[:k] = sel
        g_parts.append(g)
        r_parts.append(rr)
        e_parts.append(e)
    tokens = {
        "g": np.concatenate(g_parts),
        "rr": np.concatenate(r_parts),
        "e": np.concatenate(e_parts),
    }
    return chunks, tokens


def _pack_inputs(tokens, values_be, chunks):
    g_cols, r_cols, v_cols = [], [], []
    off = 0
    for _, _, n, _, _ in chunks:
        g = tokens["g"][off : off + n]
        rr = tokens["rr"][off : off + n]
        e = tokens["e"][off : off + n]
        off += n
        g_cols.append(g.reshape(-1, 16).T)
        r_cols.append(rr.reshape(-1, 128).T)
        v = np.zeros((n, BATCH), np.float32)
        real = e >= 0
        v[real] = values_be[:, e[real]].T
        v_cols.append(v.reshape(-1, 128, BATCH).transpose(1, 0, 2))
    g_idx = np.tile(np.concatenate(g_cols, axis=1), (8, 1)).astype(np.int16)
    relrow = np.concatenate(r_cols, axis=1).astype(BF16)
    vals = np.ascontiguousarray(np.concatenate(v_cols, axis=1).astype(BF16))
    return {
        "g_idx": np.ascontiguousarray(g_idx),
        "relrow": np.ascontiguousarray(relrow),
        "vals": vals,
    }


# A single dma_gather instruction must fit in the SWDGE descriptor ring
# (dynamic_dma_scratch_size // 16 descriptors). Split large gathers into
# sub-instructions of at most GCAP tokens.
GCAP = 1024
DMA_SCRATCH = 65536  # bytes/partition -> 4096-descriptor ring


# ---------------------------------------------------------------- device code
def _build(chunks):
    import concourse.bacc as bacc
    import concourse.bass as bass
    import concourse.mybir as mybir
    import concourse.tile as tile

    f32 = mybir.dt.float32
    bf16 = mybir.dt.bfloat16
    i16 = mybir.dt.int16
    T = sum(c[2] for c in chunks)
    S_total, C_total = T // 16, T // 128
    R = ROWS_PER_CORE

    nc = bacc.Bacc(
        None, target_bir_lowering=False, dynamic_dma_scratch_size=DMA_SCRATCH
    )
    b_t = nc.dram_tensor("b_t", [N_NODES, ELEM], bf16, kind="ExternalInput")
    g_idx = nc.dram_tensor("g_idx", [128, S_total], i16, kind="ExternalInput")
    relrow = nc.dram_tensor("relrow", [128, C_total], bf16, kind="ExternalInput")
    vals = nc.dram_tensor("vals", [128, C_total, BATCH], bf16, kind="ExternalInput")
    out_t = nc.dram_tensor("out_t", [R, ELEM], f32, kind="ExternalOutput")

    n_cols_of_window = {}
    for w, _, n, _, _ in chunks:
        n_cols_of_window[w] = n_cols_of_window.get(w, 0) + n // 128

    with tile.TileContext(nc) as tc:
        with (
            tc.tile_pool(name="gt", bufs=3) as gp,
            tc.tile_pool(name="aux", bufs=6) as auxp,
            tc.tile_pool(name="oh", bufs=3) as ohp,
            tc.tile_pool(name="ot", bufs=3) as otp,
            tc.tile_pool(name="psum", bufs=6, space="PSUM") as psp,
            tc.tile_pool(name="const", bufs=1) as cp,
        ):
            iota = cp.tile([128, 128], bf16)
            nc.gpsimd.iota(
                iota[:], pattern=[[1, 128]], base=0, channel_multiplier=0,
                allow_small_or_imprecise_dtypes=True,
            )

            off = 0
            acc = None
            col_of_window = 0
            for w, bank_b, n, first, last in chunks:
                S, C = n // 16, n // 128
                so, co = off // 16, off // 128
                off += n
                gi = auxp.tile([128, S], i16, tag="gi")
                rr = auxp.tile([128, C], bf16, tag="rr")
                vt = auxp.tile([128, C, BATCH], bf16, tag="vt")
                nc.sync.dma_start(gi[:], g_idx[:, so : so + S])
                nc.sync.dma_start(rr[:], relrow[:, co : co + C])
                nc.sync.dma_start(vt[:], vals[:, co : co + C])

                gt = gp.tile([128, C, ELEM], bf16, tag="gt")
                src = b_t[0:BANK] if not bank_b else b_t[BANK:N_NODES]
                for c0 in range(0, C, GCAP // 128):
                    c1 = min(c0 + GCAP // 128, C)
                    nsub = (c1 - c0) * 128
                    nc.gpsimd.dma_gather(
                        gt[:, c0:c1, :], src,
                        gi[:, c0 * 8 : c0 * 8 + nsub // 16],
                        nsub, nsub, ELEM,
                    )

                # scale gathered rows by per-(token, batch) edge values in one
                # DVE op: gt[p, c, k*64+f] *= vt[p, c, k]
                g_ap = gt[:]
                g3 = bass.AP(
                    g_ap.tensor, g_ap.offset,
                    [g_ap.ap[0], [ELEM, C], [FEAT, BATCH], [1, FEAT]],
                )
                v_ap = vt[:]
                v3 = bass.AP(
                    v_ap.tensor, v_ap.offset,
                    [v_ap.ap[0], [BATCH, C], [1, BATCH], [0, FEAT]],
                )
                nc.vector.tensor_mul(g3, g3, v3)

                # one-hot lhsT for all C columns in one DVE op:
                # oh[p, c, m] = (iota[p, m] == rr[p, c])
                oh = ohp.tile([128, C, 128], bf16, tag="oh")
                i_ap = iota[:]
                i3 = bass.AP(
                    i_ap.tensor, i_ap.offset, [i_ap.ap[0], [0, C], [1, 128]]
                )
                r_ap = rr[:]
                r3 = bass.AP(
                    r_ap.tensor, r_ap.offset, [r_ap.ap[0], [1, C], [0, 128]]
                )
                nc.vector.tensor_tensor(
                    oh[:], i3, r3, mybir.AluOpType.is_equal
                )

                if first:
                    acc = psp.tile([128, ELEM], f32, tag="acc")
                    col_of_window = 0
                for c in range(C):
                    nc.tensor.matmul(
                        acc[:], oh[:, c, :], gt[:, c, :],
                        start=(col_of_window == 0),
                        stop=(col_of_window == n_cols_of_window[w] - 1),
                    )
                    col_of_window += 1

                if last:
                    r0 = w * W
                    r1 = min(r0 + W, R)
                    ot = otp.tile([128, ELEM], f32, tag="ot")
                    nc.vector.tensor_copy(ot[:], acc[:])
                    nc.sync.dma_start(out_t[r0:r1], ot[: r1 - r0])

    nc.compile()
    return nc


# ---------------------------------------------------------------- entry point
def _prepare(indices, values):
    row = np.asarray(indices[0], np.int64)
    col = np.asarray(indices[1], np.int64)
    values = np.asarray(values, np.float32)
    chunks, tokens = _make_structure(row, col)
    packs = [_pack_inputs(tokens, values, chunks)]
    return chunks, packs


def _get_program(indices, values):
    key = (
        hashlib.sha1(np.ascontiguousarray(indices).tobytes()).hexdigest()
        + hashlib.sha1(np.ascontiguousarray(values).tobytes()).hexdigest()
    )
    if key not in _cache:
        from concourse.bass_interp import get_hw_module

        chunks, packs = _prepare(indices, values)
        nc = _build(chunks)
        hw_m = get_hw_module(nc.m)
        _cache[key] = (nc, hw_m, chunks, packs)
    return _cache[key]


def kernel(indices, values, shape_m, shape_n, b):
    import concourse.bass_utils as bass_utils

    indices = np.asarray(indices)
    b = np.asarray(b, np.float32)
    assert int(shape_m) == N_NODES and int(shape_n) == N_NODES
    assert b.shape == (BATCH, N_NODES, FEAT)

    nc, hw_m, chunks, packs = _get_program(indices, values)
    b_t = make_b_t(b)
    in_maps = [{"b_t": b_t, **packs[0]}]

    old_m = nc.m
    nc.m = hw_m
    try:
        res = bass_utils.run_bass_kernel_spmd(nc, in_maps, core_ids=[0])
    finally:
        nc.m = old_m

    o = res.results[0]["out_t"]  # [N_NODES, ELEM]
    return np.ascontiguousarray(
        o.reshape(N_NODES, BATCH, FEAT).transpose(1, 0, 2)
    )


# revision 12
# speedup vs baseline: 4.4061x; 4.4061x over previous
"""Batched COO SpMM (gnn_message_passing) on TRN2.

out[k, i, :] = sum_{e: row[e]==i} values[k, e] * b[k, col[e], :]
  indices [2, 800000] int32, values [4, 800000] f32, b [4, 50000, 64] f32.

Design: the whole problem runs in ONE device program on ONE NeuronCore.
(Multi-core SPMD dispatch is serialized by the runtime at ~1.5-4.5ms per
device execution, dwarfing the ~0.5ms/core of actual device work — one
execution of an 8x-bigger program is much faster end-to-end.)

Batch-fused element layout: b_t[node, k*64+f] = b[k, node, f] -> 1KB per
node row, so ONE dma_gather descriptor fetches an edge's source features
for all 4 batches. Edges (tokens) are grouped by 128-row output window;
per window, tokens split into col-bank sections (int16 gather index
limit), col-sorted, padded to multiples of 128.

Per chunk: dma_gather from b_t -> gt [128 tokens/col-group, C, 256];
DVE multiplies each token block by its per-batch edge values; per
128-token column DVE builds a one-hot lhsT[t, m] = (relrow[t] == m) and
PE accumulates psum[m,:] += lhsT^T @ gt[:,c,:] over the window's columns.
Window end: PSUM -> SBUF -> DMA to out_t rows. No scatter, no RMW.
Pad tokens: gather node 0, values 0, relrow -1 (one-hot row all zero).
"""
import hashlib

import ml_dtypes
import numpy as np

BF16 = ml_dtypes.bfloat16

N_NODES = 50000
NNZ = 800000
BATCH = 4
FEAT = 64
ELEM = BATCH * FEAT
N_CORES = 1  # whole problem on one core (see module docstring)
ROWS_PER_CORE = N_NODES
BANK = 32768
W = 128  # output rows per PSUM window

_cache = {}


def make_b_t(b):
    """Host-side packing of b into the kernel's batch-fused bf16 layout."""
    b = np.asarray(b, np.float32)
    return np.ascontiguousarray(
        b.transpose(1, 0, 2).reshape(N_NODES, ELEM).astype(BF16)
    )


# ---------------------------------------------------------------- host prep
def _make_structure(rows, cols):
    """Group edges by 128-row output window; per window split into col banks
    (int16 index limit), col-sort, pad to multiples of 128."""
    NW = -(-ROWS_PER_CORE // W)
    win = rows // W
    order = np.argsort(win, kind="stable")
    bounds = np.searchsorted(win[order], np.arange(NW + 1))
    sections = []
    for w in range(NW):
        in_w = order[bounds[w] : bounds[w + 1]]
        cw = cols[in_w]
        a = in_w[cw < BANK]
        b = in_w[cw >= BANK]
        a = a[np.argsort(cols[a], kind="stable")]
        b = b[np.argsort(cols[b], kind="stable")]
        sections.append((a, b))

    # one chunk per window: bank-A tokens (padded to x128), then bank-B
    # (padded to x128); nA/nB are the per-bank padded sizes.
    chunks = []
    g_parts, r_parts, e_parts = [], [], []
    for w in range(NW):
        nA = max(-(-len(sections[w][0]) // 128) * 128, 128)
        nB = -(-len(sections[w][1]) // 128) * 128
        chunks.append((w, nA, nB))
        for bank_b, n in ((0, nA), (1, nB)):
            if n == 0:
                continue
            sel = sections[w][bank_b]
            k = len(sel)
            g = np.zeros(n, np.int16)
            rr = np.full(n, -1.0, np.float32)
            e = np.full(n, -1, np.int64)
            g[:k] = (cols[sel] - (BANK if bank_b else 0)).astype(np.int16)
            rr[:k] = (rows[sel] - w * W).astype(np.float32)
            e[:k] = sel
            g_parts.append(g)
            r_parts.append(rr)
            e_parts.append(e)
    tokens = {
        "g": np.concatenate(g_parts),
        "rr": np.concatenate(r_parts),
        "e": np.concatenate(e_parts),
    }
    return chunks, tokens


def _pack_inputs(tokens, values_be, chunks):
    g = tokens["g"]
    rr = tokens["rr"]
    e = tokens["e"]
    g_idx = np.tile(g.reshape(-1, 16).T, (8, 1)).astype(np.int16)
    relrow = rr.reshape(-1, 128).T.astype(BF16)
    v = np.zeros((len(e), BATCH), np.float32)
    real = e >= 0
    v[real] = values_be[:, e[real]].T
    vals = v.reshape(-1, 128, BATCH).transpose(1, 0, 2).astype(BF16)
    return {
        "g_idx": np.ascontiguousarray(g_idx),
        "relrow": np.ascontiguousarray(relrow),
        "vals": np.ascontiguousarray(vals),
    }


# A single dma_gather instruction with more than ~1300 descriptors crashes
# the core (NRT_EXEC_UNIT_UNRECOVERABLE); split into sub-instructions of at
# most GCAP tokens. Gathers are descriptor-rate-bound (~8ns/desc/queue);
# round-robin across NQ SWDGE queues parallelizes them (~2.6x at NQ=4).
GCAP = 1024
DMA_SCRATCH = 65536  # bytes/partition -> 4096-descriptor ring
NQ = 4
AUXGRP = 48  # windows per grouped gi/rr/vals load


# ---------------------------------------------------------------- device code
def _build(chunks):
    import concourse.bacc as bacc
    import concourse.bass as bass
    import concourse.mybir as mybir
    import concourse.tile as tile

    f32 = mybir.dt.float32
    bf16 = mybir.dt.bfloat16
    i16 = mybir.dt.int16
    T = sum(nA + nB for _, nA, nB in chunks)
    S_total, C_total = T // 16, T // 128
    R = ROWS_PER_CORE

    nc = bacc.Bacc(
        None, target_bir_lowering=False, dynamic_dma_scratch_size=DMA_SCRATCH,
        num_swdge_queues=NQ,
    )
    b_t = nc.dram_tensor("b_t", [N_NODES, ELEM], bf16, kind="ExternalInput")
    g_idx = nc.dram_tensor("g_idx", [128, S_total], i16, kind="ExternalInput")
    relrow = nc.dram_tensor("relrow", [128, C_total], bf16, kind="ExternalInput")
    vals = nc.dram_tensor("vals", [128, C_total, BATCH], bf16, kind="ExternalInput")
    out_t = nc.dram_tensor("out_t", [R, ELEM], f32, kind="ExternalOutput")

    # group chunks for batched gi/rr/vals loads
    groups = [chunks[i : i + AUXGRP] for i in range(0, len(chunks), AUXGRP)]

    with tile.TileContext(nc) as tc:
        with (
            tc.tile_pool(name="gt", bufs=4) as gp,
            tc.tile_pool(name="aux", bufs=2) as auxp,
            tc.tile_pool(name="oh", bufs=3) as ohp,
            tc.tile_pool(name="ot", bufs=3) as otp,
            tc.tile_pool(name="psum", bufs=6, space="PSUM") as psp,
            tc.tile_pool(name="const", bufs=1) as cp,
        ):
            iota = cp.tile([128, 128], bf16)
            nc.gpsimd.iota(
                iota[:], pattern=[[1, 128]], base=0, channel_multiplier=0,
                allow_small_or_imprecise_dtypes=True,
            )

            qn = 0
            off = 0
            for grp in groups:
                gn = sum(nA + nB for _, nA, nB in grp)
                gS, gC = gn // 16, gn // 128
                so, co = off // 16, off // 128
                gi = auxp.tile([128, gS], i16, tag="gi")
                rr = auxp.tile([128, gC], bf16, tag="rr")
                vt = auxp.tile([128, gC, BATCH], bf16, tag="vt")
                nc.sync.dma_start(gi[:], g_idx[:, so : so + gS])
                nc.sync.dma_start(rr[:], relrow[:, co : co + gC])
                nc.sync.dma_start(vt[:], vals[:, co : co + gC])

                goff = 0  # token offset within group
                for w, nA, nB in grp:
                    n = nA + nB
                    C = n // 128
                    gt = gp.tile([128, C, ELEM], bf16, tag="gt")
                    for bank_b, b0, b1 in ((0, 0, nA), (1, nA, n)):
                        src = b_t[0:BANK] if not bank_b else b_t[BANK:N_NODES]
                        for t0 in range(b0, b1, GCAP):
                            t1 = min(t0 + GCAP, b1)
                            nsub = t1 - t0
                            nc.gpsimd.dma_gather(
                                gt[:, t0 // 128 : t1 // 128, :], src,
                                gi[:, (goff + t0) // 16 : (goff + t1) // 16],
                                nsub, nsub, ELEM,
                                queue_num=qn % NQ,
                            )
                            qn += 1

                    # scale gathered rows by per-(token, batch) edge values in
                    # one DVE op: gt[p, c, k*64+f] *= vt[p, gco+c, k]
                    gco = goff // 128
                    g_ap = gt[:]
                    g3 = bass.AP(
                        g_ap.tensor, g_ap.offset,
                        [g_ap.ap[0], [ELEM, C], [FEAT, BATCH], [1, FEAT]],
                    )
                    v_ap = vt[:, gco : gco + C]
                    v3 = bass.AP(
                        v_ap.tensor, v_ap.offset,
                        [v_ap.ap[0], [BATCH, C], [1, BATCH], [0, FEAT]],
                    )
                    nc.vector.tensor_mul(g3, g3, v3)

                    # one-hot lhsT for all C columns in one DVE op:
                    # oh[p, c, m] = (iota[p, m] == rr[p, gco+c])
                    oh = ohp.tile([128, C, 128], bf16, tag="oh")
                    i_ap = iota[:]
                    i3 = bass.AP(
                        i_ap.tensor, i_ap.offset, [i_ap.ap[0], [0, C], [1, 128]]
                    )
                    r_ap = rr[:, gco : gco + C]
                    r3 = bass.AP(
                        r_ap.tensor, r_ap.offset, [r_ap.ap[0], [1, C], [0, 128]]
                    )
                    nc.vector.tensor_tensor(
                        oh[:], i3, r3, mybir.AluOpType.is_equal
                    )

                    acc = psp.tile([128, ELEM], f32, tag="acc")
                    for c in range(C):
                        nc.tensor.matmul(
                            acc[:], oh[:, c, :], gt[:, c, :],
                            start=(c == 0),
                            stop=(c == C - 1),
                        )

                    r0 = w * W
                    r1 = min(r0 + W, R)
                    ot = otp.tile([128, ELEM], f32, tag="ot")
                    nc.vector.tensor_copy(ot[:], acc[:])
                    nc.sync.dma_start(out_t[r0:r1], ot[: r1 - r0])
                    goff += n
                off += gn

    nc.compile()
    return nc


# ---------------------------------------------------------------- entry point
def _prepare(indices, values):
    row = np.asarray(indices[0], np.int64)
    col = np.asarray(indices[1], np.int64)
    values = np.asarray(values, np.float32)
    chunks, tokens = _make_structure(row, col)
    packs = [_pack_inputs(tokens, values, chunks)]
    return chunks, packs


def _get_program(indices, values):
    key = (
        hashlib.sha1(np.ascontiguousarray(indices).tobytes()).hexdigest()
        + hashlib.sha1(np.ascontiguousarray(values).tobytes()).hexdigest()
    )
    if key not in _cache:
        from concourse.bass_interp import get_hw_module

        chunks, packs = _prepare(indices, values)
        nc = _build(chunks)
        hw_m = get_hw_module(nc.m)
        _cache[key] = (nc, hw_m, chunks, packs)
    return _cache[key]


def kernel(indices, values, shape_m, shape_n, b):
    import concourse.bass_utils as bass_utils

    indices = np.asarray(indices)
    b = np.asarray(b, np.float32)
    assert int(shape_m) == N_NODES and int(shape_n) == N_NODES
    assert b.shape == (BATCH, N_NODES, FEAT)

    nc, hw_m, chunks, packs = _get_program(indices, values)
    b_t = make_b_t(b)
    in_maps = [{"b_t": b_t, **packs[0]}]

    old_m = nc.m
    nc.m = hw_m
    try:
        res = bass_utils.run_bass_kernel_spmd(nc, in_maps, core_ids=[0])
    finally:
        nc.m = old_m

    o = res.results[0]["out_t"]  # [N_NODES, ELEM]
    return np.ascontiguousarray(
        o.reshape(N_NODES, BATCH, FEAT).transpose(1, 0, 2)
    )


# revision 32
# speedup vs baseline: 4.5230x; 1.0265x over previous
"""Batched COO SpMM (gnn_message_passing) on TRN2.

out[k, i, :] = sum_{e: row[e]==i} values[k, e] * b[k, col[e], :]
  indices [2, 800000] int32, values [4, 800000] f32, b [4, 50000, 64] f32.

Design: the whole problem runs in ONE device program on ONE NeuronCore.
(Multi-core SPMD dispatch is serialized by the runtime at ~1.5-4.5ms per
device execution, dwarfing the ~0.5ms/core of actual device work — one
execution of an 8x-bigger program is much faster end-to-end.)

Batch-fused element layout: b_t[node, k*64+f] = b[k, node, f] -> 1KB per
node row, so ONE dma_gather descriptor fetches an edge's source features
for all 4 batches. Edges (tokens) are grouped by 128-row output window;
per window, tokens split into col-bank sections (int16 gather index
limit), col-sorted, padded to multiples of 128.

Per chunk: dma_gather from b_t -> gt [128 tokens/col-group, C, 256];
DVE multiplies each token block by its per-batch edge values; per
128-token column DVE builds a one-hot lhsT[t, m] = (relrow[t] == m) and
PE accumulates psum[m,:] += lhsT^T @ gt[:,c,:] over the window's columns.
Window end: PSUM -> SBUF -> DMA to out_t rows. No scatter, no RMW.
Pad tokens: gather node 0, values 0, relrow -1 (one-hot row all zero).
"""
import hashlib

import ml_dtypes
import numpy as np

BF16 = ml_dtypes.bfloat16

N_NODES = 50000
NNZ = 800000
BATCH = 4
FEAT = 64
ELEM = BATCH * FEAT
N_CORES = 1  # whole problem on one core (see module docstring)
ROWS_PER_CORE = N_NODES
BANK = 32768
W = 128  # output rows per PSUM window

_cache = {}


def make_b_t(b):
    """Host-side packing of b into the kernel's batch-fused bf16 layout."""
    b = np.asarray(b, np.float32)
    return np.ascontiguousarray(
        b.transpose(1, 0, 2).reshape(N_NODES, ELEM).astype(BF16)
    )


# ---------------------------------------------------------------- host prep
def _make_structure(rows, cols):
    """Group edges by 128-row output window; per window split into col banks
    (int16 index limit), col-sort, pad to multiples of 128."""
    NW = -(-ROWS_PER_CORE // W)
    win = rows // W
    order = np.argsort(win, kind="stable")
    bounds = np.searchsorted(win[order], np.arange(NW + 1))
    sections = []
    for w in range(NW):
        in_w = order[bounds[w] : bounds[w + 1]]
        cw = cols[in_w]
        a = in_w[cw < BANK]
        b = in_w[cw >= BANK]
        a = a[np.argsort(cols[a], kind="stable")]
        b = b[np.argsort(cols[b], kind="stable")]
        sections.append((a, b))

    # one chunk per window: bank-A tokens (padded to x128), then bank-B
    # (padded to x128); nA/nB are the per-bank padded sizes.
    chunks = []
    g_parts, r_parts, e_parts = [], [], []
    for w in range(NW):
        kA, kB = len(sections[w][0]), len(sections[w][1])
        nA = max(-(-kA // 128) * 128, 128)
        nB = -(-kB // 128) * 128
        chunks.append((w, nA, nB, kA, kB))
        for bank_b, n in ((0, nA), (1, nB)):
            if n == 0:
                continue
            sel = sections[w][bank_b]
            k = len(sel)
            g = np.zeros(n, np.int16)
            rr = np.full(n, -1.0, np.float32)
            e = np.full(n, -1, np.int64)
            g[:k] = (cols[sel] - (BANK if bank_b else 0)).astype(np.int16)
            rr[:k] = (rows[sel] - w * W).astype(np.float32)
            e[:k] = sel
            g_parts.append(g)
            r_parts.append(rr)
            e_parts.append(e)
    tokens = {
        "g": np.concatenate(g_parts),
        "rr": np.concatenate(r_parts),
        "e": np.concatenate(e_parts),
    }
    return chunks, tokens


def _pack_inputs(tokens, values_be, chunks):
    g = tokens["g"]
    rr = tokens["rr"]
    e = tokens["e"]
    g_idx = np.tile(g.reshape(-1, 16).T, (8, 1)).astype(np.int16)
    relrow = rr.reshape(-1, 128).T.astype(BF16)
    v = np.zeros((len(e), BATCH), np.float32)
    real = e >= 0
    v[real] = values_be[:, e[real]].T
    vals = v.reshape(-1, 128, BATCH).transpose(1, 0, 2).astype(BF16)
    return {
        "g_idx": np.ascontiguousarray(g_idx),
        "relrow": np.ascontiguousarray(relrow),
        "vals": np.ascontiguousarray(vals),
    }


# A single dma_gather instruction with more than ~1300 descriptors crashes
# the core (NRT_EXEC_UNIT_UNRECOVERABLE); split into sub-instructions of at
# most GCAP tokens. Gathers are descriptor-rate-bound (~8ns/desc/queue);
# round-robin across NQ SWDGE queues parallelizes them (~2.6x at NQ=4).
GCAP = 1024
DMA_SCRATCH = 65536  # bytes/partition -> 4096-descriptor ring
NQ = 4
AUXGRP = 48  # windows per grouped gi/rr/vals load


# ---------------------------------------------------------------- device code
def _build(chunks):
    import concourse.bacc as bacc
    import concourse.bass as bass
    import concourse.mybir as mybir
    import concourse.tile as tile

    f32 = mybir.dt.float32
    bf16 = mybir.dt.bfloat16
    i16 = mybir.dt.int16
    T = sum(c[1] + c[2] for c in chunks)
    S_total, C_total = T // 16, T // 128
    R = ROWS_PER_CORE

    nc = bacc.Bacc(
        None, target_bir_lowering=False, dynamic_dma_scratch_size=DMA_SCRATCH,
        num_swdge_queues=NQ,
    )
    b_t = nc.dram_tensor("b_t", [N_NODES, ELEM], bf16, kind="ExternalInput")
    g_idx = nc.dram_tensor("g_idx", [128, S_total], i16, kind="ExternalInput")
    relrow = nc.dram_tensor("relrow", [128, C_total], bf16, kind="ExternalInput")
    vals = nc.dram_tensor("vals", [128, C_total, BATCH], bf16, kind="ExternalInput")
    out_t = nc.dram_tensor("out_t", [R, ELEM], f32, kind="ExternalOutput")

    # group chunks for batched gi/rr/vals loads
    groups = [chunks[i : i + AUXGRP] for i in range(0, len(chunks), AUXGRP)]
    CMAX = max((c[1] + c[2]) // 128 for c in chunks)
    GT_BUFS = 4

    with tile.TileContext(nc) as tc:
        with (
            tc.tile_pool(name="gt", bufs=GT_BUFS) as gp,
            tc.tile_pool(name="aux", bufs=2) as auxp,
            tc.tile_pool(name="oh", bufs=3) as ohp,
            tc.tile_pool(name="ot", bufs=3) as otp,
            tc.tile_pool(name="psum", bufs=6, space="PSUM") as psp,
            tc.tile_pool(name="const", bufs=1) as cp,
        ):
            iota = cp.tile([128, 128], bf16)
            nc.gpsimd.iota(
                iota[:], pattern=[[1, 128]], base=0, channel_multiplier=0,
                allow_small_or_imprecise_dtypes=True,
            )


            qn = 0
            off = 0
            for grp in groups:
                gn = sum(c[1] + c[2] for c in grp)
                gS, gC = gn // 16, gn // 128
                so, co = off // 16, off // 128
                gi = auxp.tile([128, gS], i16, tag="gi")
                rr = auxp.tile([128, gC], bf16, tag="rr")
                vt = auxp.tile([128, gC, BATCH], bf16, tag="vt")
                nc.sync.dma_start(gi[:], g_idx[:, so : so + gS])
                nc.sync.dma_start(rr[:], relrow[:, co : co + gC])
                nc.sync.dma_start(vt[:], vals[:, co : co + gC])

                goff = 0  # token offset within group
                for w, nA, nB, kA, kB in grp:
                    n = nA + nB
                    C = n // 128
                    gt = gp.tile([128, C, ELEM], bf16, tag="gt")
                    for bank_b, b0, b1, k in ((0, 0, nA, kA), (1, nA, n, kB)):
                        src = b_t[0:BANK] if not bank_b else b_t[BANK:N_NODES]
                        for t0 in range(b0, b1, GCAP):
                            t1 = min(t0 + GCAP, b1)
                            nsub = t1 - t0
                            nc.gpsimd.dma_gather(
                                gt[:, t0 // 128 : t1 // 128, :], src,
                                gi[:, (goff + t0) // 16 : (goff + t1) // 16],
                                nsub, nsub, ELEM,
                                queue_num=qn % NQ,
                            )
                            qn += 1

                    # scale gathered rows by per-(token, batch) edge values in
                    # one DVE op: gt[p, c, k*64+f] *= vt[p, gco+c, k]
                    gco = goff // 128
                    g_ap = gt[:]
                    g3 = bass.AP(
                        g_ap.tensor, g_ap.offset,
                        [g_ap.ap[0], [ELEM, C], [FEAT, BATCH], [1, FEAT]],
                    )
                    v_ap = vt[:, gco : gco + C]
                    v3 = bass.AP(
                        v_ap.tensor, v_ap.offset,
                        [v_ap.ap[0], [BATCH, C], [1, BATCH], [0, FEAT]],
                    )
                    nc.vector.tensor_mul(g3, g3, v3)

                    # one-hot lhsT for all C columns in one DVE op:
                    # oh[p, c, m] = (iota[p, m] == rr[p, gco+c])
                    oh = ohp.tile([128, C, 128], bf16, tag="oh")
                    i_ap = iota[:]
                    i3 = bass.AP(
                        i_ap.tensor, i_ap.offset, [i_ap.ap[0], [0, C], [1, 128]]
                    )
                    r_ap = rr[:, gco : gco + C]
                    r3 = bass.AP(
                        r_ap.tensor, r_ap.offset, [r_ap.ap[0], [1, C], [0, 128]]
                    )
                    nc.vector.tensor_tensor(
                        oh[:], i3, r3, mybir.AluOpType.is_equal
                    )

                    acc = psp.tile([128, ELEM], f32, tag="acc")
                    for c in range(C):
                        nc.tensor.matmul(
                            acc[:], oh[:, c, :], gt[:, c, :],
                            start=(c == 0),
                            stop=(c == C - 1),
                        )

                    r0 = w * W
                    r1 = min(r0 + W, R)
                    ot = otp.tile([128, ELEM], f32, tag="ot")
                    nc.scalar.activation(
                        ot[:], acc[:], mybir.ActivationFunctionType.Copy
                    )
                    nc.sync.dma_start(out_t[r0:r1], ot[: r1 - r0])
                    goff += n
                off += gn

    nc.compile()
    return nc


# ---------------------------------------------------------------- entry point
def _prepare(indices, values):
    row = np.asarray(indices[0], np.int64)
    col = np.asarray(indices[1], np.int64)
    values = np.asarray(values, np.float32)
    chunks, tokens = _make_structure(row, col)
    packs = [_pack_inputs(tokens, values, chunks)]
    return chunks, packs


def _get_program(indices, values):
    key = (
        hashlib.sha1(np.ascontiguousarray(indices).tobytes()).hexdigest()
        + hashlib.sha1(np.ascontiguousarray(values).tobytes()).hexdigest()
    )
    if key not in _cache:
        from concourse.bass_interp import get_hw_module

        chunks, packs = _prepare(indices, values)
        nc = _build(chunks)
        hw_m = get_hw_module(nc.m)
        _cache[key] = (nc, hw_m, chunks, packs)
    return _cache[key]


def kernel(indices, values, shape_m, shape_n, b):
    import concourse.bass_utils as bass_utils

    indices = np.asarray(indices)
    b = np.asarray(b, np.float32)
    assert int(shape_m) == N_NODES and int(shape_n) == N_NODES
    assert b.shape == (BATCH, N_NODES, FEAT)

    nc, hw_m, chunks, packs = _get_program(indices, values)
    b_t = make_b_t(b)
    in_maps = [{"b_t": b_t, **packs[0]}]

    old_m = nc.m
    nc.m = hw_m
    try:
        res = bass_utils.run_bass_kernel_spmd(nc, in_maps, core_ids=[0])
    finally:
        nc.m = old_m

    o = res.results[0]["out_t"]  # [N_NODES, ELEM]
    return np.ascontiguousarray(
        o.reshape(N_NODES, BATCH, FEAT).transpose(1, 0, 2)
    )
